# revision 11
# baseline (speedup 1.0000x reference)
"""Trainium2 Bass kernel for nn_BasicRNN (2-layer LSTM, H=32, S=64, B=8192).

Strategy: pure data parallel over 8 cores (1024 batch each). Per core the
batch is laid out in "T-layout" tiles [128 partitions = 4 groups x 32
features, 256 batch (free)]; the 256-batch free dim is split into four
64-wide chunks whose dependency chains software-pipeline across the
engines (four phase-shifted chains keep ScalarE/DVE busy while each
chunk's serial recurrence closes; this sits right at the ScalarE
busy-time floor of ~2.55us per step).

The wall-clock is bound by the 128-step serial recurrence (64 steps x 2
layers); per chunk-step the critical cycle is
    h -> 4 recurrent matmuls -> tanh(gates) -> u,v -> c* -> tanh(c*) -> h
HW-measured notes (invisible in the cost-model sim): every stationary
weight switch costs ~100ns on the PE path, so input projections for step
k+1 are emitted at the TOP of step k, gate-major, with each loaded weight
serving all four chunks; x DMA prefetch distance is 2.

Sigmoids use the tanh identity sigmoid(x) = (1 + tanh(x/2))/2 with scale
factors folded into host-prepped weights; cell and hidden state are stored
doubled (c* = 2c, h* = 2h):
    u  = (tanh_f + 1) * c*          v = (tanh_i + 1) * tanh_g
    c* = 0.5*u + v                  h* = (tanh_o + 1) * tanh(0.5 * c*)
"""
import sys
sys.path.insert(0, '/opt/trn_rl_repo')

import numpy as np

import concourse.bacc as bacc
import concourse.tile as tile
from concourse import mybir
from concourse.bass_utils import run_bass_kernel_spmd

F32 = mybir.dt.float32
F16 = mybir.dt.float16
TANH = mybir.ActivationFunctionType.Tanh
IDENT = mybir.ActivationFunctionType.Identity
ADD = mybir.AluOpType.add
MULT = mybir.AluOpType.mult

B, S, NX, NSFC, H, NY = 8192, 64, 4, 5, 32, 1
NCORES = 8
BC = B // NCORES
NG = 4
GB = BC // NG
CHUNKS = [(0, 64), (64, 64), (128, 64), (192, 64)]
GATES = [("g", 2 * H, 1.0), ("f", H, 0.5), ("i", 0, 0.5), ("o", 3 * H, 0.5)]

def _prep_weights(inp):
    w = {}

    def blockdiag(wmat, scale_fio, scale_g):
        t = np.zeros((128, 512), np.float32)
        for gi, (nm, r0, _) in enumerate(GATES):
            s = scale_g if nm == "g" else scale_fio
            blk = (wmat[r0:r0 + H] * s).T.astype(np.float32)
            for g in range(NG):
                t[32 * g:32 * g + H,
                  128 * gi + 32 * g:128 * gi + 32 * g + 32] = blk
        return t.astype(np.float16)

    t = np.zeros((20, 512), np.float32)
    btot1 = inp["b_ih1"] + inp["b_hh1"]
    for gi, (nm, r0, trick) in enumerate(GATES):
        s = 1.0 if nm == "g" else 0.5
        blk = (inp["w_ih1"][r0:r0 + H] * s).T.astype(np.float32)
        for g in range(NG):
            c0 = 128 * gi + 32 * g
            t[5 * g:5 * g + NX, c0:c0 + 32] = blk
            t[5 * g + NX, c0:c0 + 32] = btot1[r0:r0 + H] * trick
    w["WX1"] = t.astype(np.float16)

    w["WL1"] = blockdiag(inp["w_hh1"], 0.25, 0.5)
    w["WX2"] = blockdiag(inp["w_ih2"], 0.25, 0.5)
    w["WL2"] = blockdiag(inp["w_hh2"], 0.25, 0.5)

    # layer-2 bias matmul: B2 [128, 128] as before; ONESB [128, 4, 256]:
    # onesb[32g+a, a, :] = 1.0 (gate-major blocks sliced per chunk).
    b2 = np.zeros((128, 128), np.float32)
    onesb = np.zeros((128, 4, 256), np.float32)
    btot2 = inp["b_ih2"] + inp["b_hh2"]
    for gi, (_, r0, trick) in enumerate(GATES):
        for g in range(NG):
            b2[32 * g + gi, 32 * g:32 * g + 32] = btot2[r0:r0 + H] * trick
            onesb[32 * g + gi, gi, :] = 1.0
    w["B2"] = b2.astype(np.float16)
    w["ONESB"] = onesb.astype(np.float16)

    ws = np.zeros((8, 64), np.float32)
    ws[:NSFC, 0:32] = inp["w_sfc1"].T
    ws[:NSFC, 32:64] = inp["w_sfc2"].T
    w["WSFC"] = ws
    bs = np.zeros((128, 2), np.float32)
    for g in range(NG):
        bs[32 * g:32 * g + 32, 0] = inp["b_sfc1"]
        bs[32 * g:32 * g + 32, 1] = inp["b_sfc2"]
    w["BSFC"] = bs

    wo = np.zeros((128, 1), np.float32)
    for g in range(NG):
        wo[32 * g:32 * g + 32, 0] = inp["w_out"][0] * 0.5
    w["WOUT"] = wo.astype(np.float16)
    w["BOUT"] = np.full((128, 1), float(inp["b_out"][0]), np.float32)
    return w



_CACHED = {}


def build_program(n_steps=S, trace_sim=False, reps=0, split_o=False):
    split_o = split_o in (True, 'True')
    nc = bacc.Bacc()
    d = {}
    d["xs"] = nc.declare_dram_parameter("xs", [n_steps, 20, GB], F16,
                                        isOutput=False)
    d["sfcT"] = nc.declare_dram_parameter("sfcT", [8, BC], F32, isOutput=False)
    F16W = {"WX1", "WL1", "WX2", "WL2", "B2", "ONESB", "WOUT"}
    WSHAPES = [("WX1", [20, 512]), ("WL1", [128, 512]),
               ("WX2", [128, 512]), ("WL2", [128, 512]),
               ("B2", [128, 128]), ("ONESB", [128, 4, 256]),
               ("WSFC", [8, 64]), ("BSFC", [128, 2]),
               ("WOUT", [128, 1]), ("BOUT", [128, 1])]
    for nm, shape in WSHAPES:
        d[nm] = nc.declare_dram_parameter(nm, shape,
                                          F16 if nm in F16W else F32,
                                          isOutput=False)
    y_out = nc.declare_dram_parameter("y", [NG, n_steps * GB], F32,
                                      isOutput=True)

    NS1 = n_steps + 1
    NCH = len(CHUNKS)

    with tile.TileContext(nc, trace_sim=trace_sim) as tc:
        with tc.tile_pool(name="wpool", bufs=1) as wpool, \
             tc.tile_pool(name="big", bufs=1) as big, \
             tc.tile_pool(name="work", bufs=4) as work, \
             tc.tile_pool(name="xp", bufs=4) as xp, \
             tc.tile_pool(name="yp", bufs=2) as yp, \
             tc.tile_pool(name="psA", bufs=2, space="PSUM") as psA:

            W = {}
            for nm, shape in WSHAPES:
                t = wpool.tile(shape, F16 if nm in F16W else F32, tag=nm)
                nc.sync.dma_start(t[:], d[nm][:])
                W[nm] = t
            sfcT = wpool.tile([8, BC], F32, tag="sfcT")
            nc.sync.dma_start(sfcT[:], d["sfcT"][:])

            h1_all = big.tile([128, NS1 * GB], F16, tag="h1_all")
            h2_all = big.tile([128, NS1 * GB], F16, tag="h2_all")

            def new_CT(ch, w):
                return work.tile([128, 5, w], F16, tag=f"CT{ch}",
                                 name=f"CT{ch}")

            ph = psA.tile([128, 2 * GB], F32, tag="G0")
            for g in range(NG):
                nc.tensor.matmul(ph[32 * g:32 * g + 32, 0:GB],
                                 W["WSFC"][0:NSFC, 0:32],
                                 sfcT[0:NSFC, GB * g:GB * (g + 1)],
                                 start=True, stop=True,
                                 tile_position=(0, 32 * g))
                nc.tensor.matmul(ph[32 * g:32 * g + 32, GB:2 * GB],
                                 W["WSFC"][0:NSFC, 32:64],
                                 sfcT[0:NSFC, GB * g:GB * (g + 1)],
                                 start=True, stop=True,
                                 tile_position=(0, 32 * g))
            t0 = work.tile([128, GB], F32, tag="t0")
            nc.scalar.activation(t0[:], ph[:, 0:GB], TANH, bias=W["BSFC"][:, 0:1])
            nc.vector.tensor_scalar_mul(
                h1_all[:, n_steps * GB:(n_steps + 1) * GB], t0[:], 2.0)
            t0b = work.tile([128, GB], F32, tag="t0")
            nc.scalar.activation(t0b[:], ph[:, GB:2 * GB], TANH,
                                 bias=W["BSFC"][:, 1:2])
            nc.vector.memset(h2_all[:, 0:GB], 0.0)

            def emit_inputs_all(layer, k, xstep):
                """Gate-major input projections for ALL chunks of step k."""
                Gs = [psA.tile([128, 4, w], F32, tag=f"G{ch}", name=f"G{ch}")
                      for ch, (off, w) in enumerate(CHUNKS)]
                if layer == 1:
                    for gi in range(4):
                        for ch, (off, w) in enumerate(CHUNKS):
                            nc.tensor.matmul(
                                Gs[ch][:, gi, :],
                                W["WX1"][0:20, gi * 128:(gi + 1) * 128],
                                xstep[0:20, off:off + w],
                                start=(gi == 0), stop=False)
                else:
                    for ch, (off, w) in enumerate(CHUNKS):
                        nc.tensor.matmul(Gs[ch][:, :, :], W["B2"][:, 0:128],
                                         W["ONESB"][:, :, off:off + w],
                                         start=True, stop=False)
                    for gi in range(4):
                        for ch, (off, w) in enumerate(CHUNKS):
                            nc.tensor.matmul(
                                Gs[ch][:, gi, :],
                                W["WX2"][:, gi * 128:(gi + 1) * 128],
                                h1_all[:, k * GB + off:k * GB + off + w],
                                start=False, stop=False)
                return Gs

            def scan_body(iv=None):
                for layer in (1, 2):
                    WL = W["WL1"] if layer == 1 else W["WL2"]
                    hall = h1_all if layer == 1 else h2_all

                    xq = [None, None]
                    if layer == 1:
                        xq = [xp.tile([20, GB], F16, tag="x", name="x")
                              for _ in (0, 1)]
                        nc.sync.dma_start(xq[0][:], d["xs"][0])
                        if n_steps > 1:
                            nc.sync.dma_start(xq[1][:], d["xs"][1])
                    Tcur = []
                    for ch, (off, w) in enumerate(CHUNKS):
                        t = new_CT(ch, w)
                        if layer == 1:
                            nc.vector.tensor_scalar_mul(
                                t[:, 0, :], t0b[:, off:off + w], 2.0)
                        else:
                            nc.vector.memset(t[:, 0, :], 0.0)
                        Tcur.append(t)
                    Gcur = emit_inputs_all(layer, 0, xq[0])
                    Gnext = [None] * NCH

                    for k in range(n_steps):
                        if layer == 1:
                            rhs_idx, out_idx = n_steps - k, n_steps - 1 - k
                        else:
                            rhs_idx, out_idx = k, k + 1
                        if layer == 1 and k + 2 < n_steps:
                            xfut = xp.tile([20, GB], F16, tag="x")
                            nc.sync.dma_start(xfut[:], d["xs"][k + 2])
                        else:
                            xfut = None
                        if k + 1 < n_steps:
                            Gnext = emit_inputs_all(layer, k + 1, xq[1])

                        for ch in range(NCH):
                            off, w = CHUNKS[ch]
                            G = Gcur[ch]
                            rhs = hall[:, rhs_idx * GB + off:
                                       rhs_idx * GB + off + w]
                            for gi in range(4):
                                nc.tensor.matmul(
                                    G[:, gi, :],
                                    WL[:, gi * 128:(gi + 1) * 128],
                                    rhs, start=False, stop=(gi == 3))
                            CT = Tcur[ch]
                            if split_o:
                                nc.scalar.activation(CT[:, 1:4, :],
                                                     G[:, 0:3, :], TANH)
                                nc.scalar.activation(CT[:, 4, :],
                                                     G[:, 3, :], TANH)
                            else:
                                nc.scalar.activation(CT[:, 1:5, :],
                                                     G[:, :, :], TANH)
                            CTn = new_CT(ch, w)
                            UV = work.tile([128, 2, w], F16, tag="UV")
                            nc.vector.scalar_tensor_tensor(
                                UV[:], CT[:, 2:4, :], 1.0,
                                CT[:, 0:2, :], ADD, MULT)
                            nc.vector.scalar_tensor_tensor(
                                CTn[:, 0, :], UV[:, 0, :], 0.5,
                                UV[:, 1, :], MULT, ADD)
                            TC = work.tile([128, w], F16, tag="TC")
                            nc.scalar.activation(TC[:], CTn[:, 0, :],
                                                 TANH, scale=0.5)
                            nc.vector.scalar_tensor_tensor(
                                hall[:, out_idx * GB + off:
                                     out_idx * GB + off + w],
                                CT[:, 4, :], 1.0, TC[:], ADD, MULT)
                            Tcur[ch] = CTn
                            Gcur[ch] = Gnext[ch]
                        if layer == 1:
                            xq = [xq[1], xfut]

            if reps:
                with tc.For_i(0, reps, 1) as iv:
                    scan_body(iv)
            else:
                scan_body()

            YCH = 512
            total = n_steps * GB
            nch = total // YCH
            for ci in range(nch):
                py = psA.tile([128, YCH], F32, tag="G0", name="py")
                for g in range(NG):
                    for j in range(YCH // 512):
                        off = GB + ci * YCH + j * 512
                        nc.tensor.matmul(py[32 * g:32 * g + 1,
                                            j * 512:(j + 1) * 512],
                                         W["WOUT"][32 * g:32 * g + 32, 0:1],
                                         h2_all[32 * g:32 * g + 32,
                                                off:off + 512],
                                         start=True, stop=True,
                                         tile_position=(32 * g, 32 * g))
                ysb = yp.tile([128, YCH], F32, tag="ysb")
                nc.scalar.activation(ysb[:], py[:], IDENT, bias=W["BOUT"][:, 0:1])
                for g in range(NG):
                    nc.sync.dma_start(y_out[g, ci * YCH:(ci + 1) * YCH],
                                      ysb[32 * g:32 * g + 1, :])
    nc.finalize()
    return nc


def kernel(**inputs):
    inputs = {k: np.asarray(v) for k, v in inputs.items()}
    if "nc" not in _CACHED:
        _CACHED["nc"] = build_program(S)
    nc = _CACHED["nc"]
    wts = _prep_weights(inputs)
    x = inputs["inputs_main"]
    sfc = inputs["inputs_sfc"]
    in_maps = []
    for c in range(NCORES):
        xs_c = x[c * BC:(c + 1) * BC]
        sfc_c = sfc[c * BC:(c + 1) * BC]
        xr = xs_c[:, ::-1, :]
        xg = xr.reshape(NG, GB, S, NX).transpose(2, 0, 3, 1)
        xs_arr = np.ones((S, NG, 5, GB), np.float32)
        xs_arr[:, :, :NX, :] = xg
        xs_arr = xs_arr.reshape(S, 20, GB)
        sfcT = np.zeros((8, BC), np.float32)
        sfcT[:NSFC] = sfc_c.T
        m = {"xs": xs_arr.astype(np.float16), "sfcT": sfcT}
        m.update(wts)
        in_maps.append(m)
    res = run_bass_kernel_spmd(nc, in_maps, list(range(NCORES)))
    y = np.empty((B, S, NY), np.float32)
    for c in range(NCORES):
        yc = res.results[c]["y"]
        yc = yc.reshape(NG, S, GB).transpose(0, 2, 1)
        y[c * BC:(c + 1) * BC, :, 0] = yc.reshape(BC, S)
    return y
